# revision 9
# baseline (speedup 1.0000x reference)
"""Trainium2 Bass kernel for the MANTIS quantum-circuit-loss nn.Module.

Shapes (hardcoded): B=128, L=16, M=32, P=4.  8 NeuronCores, batch-sharded
(16 batch elements per core).

Math
----
Let j = (m, p) flattened (M*P = 128 == partition count) and
    A[b, l, j] = theta[l, j] + scal[p(j)] * input_ds[b, l]
    CA = cos(A), SA = sin(A)                       (ACT Sin + pi/2 bias)

prob term:      amp[b]  = sum_j coef_j prod_l CA[b,l,j]
normalization:  norm[b] = sum_{j,k} coef_j coef_k prod_l cos(A[b,l,j]-A[b,l,k])

norm[b] is the squared norm of a sum of 128 product states in the 2^16-dim
site space.  Split the 16 sites into two groups of 8; per group build the
256 branch-product vectors U_g[j, T] by log-doubling (bf16 elementwise
multiplies, split across DVE and Pool).  With coef folded into U0 (site 0):
    D_b[T1, T2] = sum_j (c U0)[j, T1] U1[j, T2]    (PE matmul, K=128, bf16)
    norm[b] = ||D_b||^2      amp[b] = D_b[0, 0]    (read from PSUM directly)
    loss_b  = (ln(norm) - ln(amp^2)) / B           (EPS term negligible)

Squares: batch elements are grouped [2,4,3,4,3] into wide PSUM tiles; ACT
does one Square pass per group (PSUM f32 -> SBUF bf16), then one DVE
tensor_scalar per element accumulates into fin (bf16 4x mode).  The final
partition reduction is one ones-matmul.  Regularization variances run on
Pool + PE, fully overlapped.  Each core returns [1,2]:
    out[0,0] = (1/128) * sum_{local b} (ln norm_b - ln amp_b^2)
    out[0,1] = REG_C*var(coef) + REG_THETA_M*... + REG_THETA_P*...
Host combine: loss = sum_c out_c[0,0] + out_0[0,1].
"""

import os

import numpy as np

import concourse.bacc as bacc
import concourse.bass as bass
import concourse.mybir as mybir
import concourse.tile as tile

B, L, M, P = 128, 16, 32, 4
NCORES = 8
BLOC = B // NCORES  # 16 batch elements per core
J = M * P  # 128
REG_C = 0.01
REG_THETA_M = 0.01
REG_THETA_P = 0.01

F32 = mybir.dt.float32
BF16 = mybir.dt.bfloat16
AF = mybir.ActivationFunctionType
ALU = mybir.AluOpType

MM_DT = BF16 if os.environ.get("MANTIS_MM_DT", "bf16") == "bf16" else mybir.dt.float32r

# batch grouping into PSUM tiles: [2,4,3,4,3] alternating pools B,A,B,A,B
BGRP = [int(x) for x in os.environ.get("MANTIS_BGRP", "2,4,3,4,3").split(",")]
# per group: how many leading b's use ACT-direct square+accum (rest: wide+TS)
NDIR = [int(x) for x in os.environ.get("MANTIS_NDIR", "1,2,2,2,2").split(",")]
# per group: engine for the group's U-build, one char per site-group:
#   'd' = DVE, 'p' = Pool
UENG = os.environ.get("MANTIS_UENG", "dp,dp,dp,dp,dp").split(",")
# engine for the TS square-reduce per batch element (flat list over wide b's)
TSENG = os.environ.get("MANTIS_TSENG", "d" * 16)
# engine for L1/L2 doubling per site-group
LENG = os.environ.get("MANTIS_LENG", "dp")

# input blob column layout
PC_THETA = 0  # 16 cols: theta_t[j, l]
PC_COEF = 16  # 1 col
PC_SCAL = 17  # 1 col: pi / 2^(p(j)+1)
PC_DVEC = 18  # 1 col: 1/n for the var terms (rows 0:37)
PC_HALFPI = 19  # 1 col: pi/2 (ACT bias for cos-via-sin)
PC_IND = 20  # 256 cols: input_ds slice, [i, l]
PC_MASK = 276  # 37 cols: [ones | mask_p(4) | mask_m(32)]
PC_REGW = 313  # 17 cols: reg weights (rows 0:37)
IN_COLS = 330


def build_blob() -> np.ndarray:
    pr = np.zeros((J, IN_COLS), dtype=np.float32)
    sf = (np.pi / 2.0 ** (np.arange(P) + 1.0)).astype(np.float32)
    pr[:, PC_SCAL] = np.tile(sf, M)
    pr[0, PC_DVEC] = 1.0 / 128.0
    pr[1:5, PC_DVEC] = 1.0 / 32.0
    pr[5:37, PC_DVEC] = 1.0 / 4.0
    pr[:, PC_HALFPI] = np.pi / 2.0
    pr[:, PC_MASK] = 1.0  # ones
    jj = np.arange(J)
    pr[jj, PC_MASK + 1 + (jj % 4)] = 1.0  # mask_p
    pr[jj, PC_MASK + 5 + (jj // 4)] = 1.0  # mask_m
    # REGW rows 0:37: weight for each cell of (S^2/n - SS) so that
    # sum(REGW * (S^2/n - SS)) == reg_total; var = (SS - S^2/n)/(n-1).
    pr[0, PC_REGW + 16] = -REG_C / 127.0
    pr[1:5, PC_REGW : PC_REGW + 16] = -REG_THETA_M / 64.0 / 31.0
    pr[5:37, PC_REGW : PC_REGW + 16] = -REG_THETA_P / 512.0 / 3.0
    return pr


def build_program():
    nc = bacc.Bacc(
        "TRN2",
        target_bir_lowering=False,
        debug=False,
        num_devices=NCORES,
    )
    blob_d = nc.dram_tensor("blob", [J, IN_COLS], F32, kind="ExternalInput")
    out_d = nc.dram_tensor("out", [1, 2], F32, kind="ExternalOutput")

    with tile.TileContext(nc) as tc:
        with (
            tc.tile_pool(name="const", bufs=1) as cpool,
            tc.tile_pool(name="work", bufs=1) as wpool,
            tc.tile_pool(name="upl", bufs=10) as upool,
            tc.tile_pool(name="sq", bufs=2) as qpool,
            tc.tile_pool(name="pa", bufs=1, space=bass.MemorySpace.PSUM) as pa,
            tc.tile_pool(name="pb", bufs=1, space=bass.MemorySpace.PSUM) as pb,
            tc.tile_pool(name="fps", bufs=1, space=bass.MemorySpace.PSUM) as fpool,
        ):
            _emit(nc, tc, cpool, wpool, upool, qpool, pa, pb, fpool, blob_d, out_d)
    nc.compile()
    return nc


def _eng(nc, c):
    return nc.vector if c == "d" else nc.gpsimd


def _emit(nc, tc, cpool, wpool, upool, qpool, pa, pb, fpool, blob_d, out_d):
    blob = cpool.tile([J, IN_COLS], F32, tag="blob")
    # critical head (params + inds) first; masks/regw second
    nc.sync.dma_start(blob[:, 0:PC_MASK], blob_d[:, 0:PC_MASK])
    nc.sync.dma_start(blob[:, PC_MASK:], blob_d[:, PC_MASK:])

    # warm the Sin ACT table while the input DMA is in flight
    scrsin = wpool.tile([1, 1], F32, tag="scrsin")
    nc.gpsimd.memset(scrsin[:], 0.0)
    scrsin2 = wpool.tile([1, 1], F32, tag="scrsin2")
    nc.scalar.activation(scrsin2[:], scrsin[:], AF.Sin)

    theta_ap = blob[:, PC_THETA : PC_THETA + L]
    coef_ap = blob[:, PC_COEF : PC_COEF + 1]
    scal_ap = blob[:, PC_SCAL : PC_SCAL + 1]
    ones_ap = blob[:, PC_MASK : PC_MASK + 1]

    # --- stage A: ARG[j, (i,l)] = theta[j,l] + scal[j]*inds[i,l]
    arg = wpool.tile([J, BLOC * L], F32, tag="arg")
    in_bc = blob[:, PC_IND : PC_IND + BLOC * L].rearrange(
        "j (i l) -> j i l", i=BLOC, l=L
    )
    th_bc = theta_ap.unsqueeze(1).broadcast_to([J, BLOC, L])
    arg_v = arg[:].rearrange("j (i l) -> j i l", i=BLOC, l=L)
    nc.vector.scalar_tensor_tensor(
        out=arg_v, in0=in_bc, scalar=scal_ap, in1=th_bc,
        op0=ALU.mult, op1=ALU.add,
    )

    # --- CS[j, (t,i,l)] in bf16: t=0 -> cos(A), t=1 -> sin(A)
    cs = wpool.tile([J, 2 * BLOC * L], F32, tag="cs")
    nc.scalar.activation(
        cs[:, 0 : BLOC * L], arg[:], AF.Sin,
        bias=blob[:, PC_HALFPI : PC_HALFPI + 1], scale=-1.0,
    )
    nc.scalar.activation(cs[:, BLOC * L : 2 * BLOC * L], arg[:], AF.Sin)

    cs_v = cs[:].rearrange("j (t i l) -> j t i l", t=2, i=BLOC, l=L)
    # fold coef into site l=0 (both branches)
    nc.vector.tensor_scalar_mul(cs_v[:, :, :, 0:1], cs_v[:, :, :, 0:1], coef_ap)

    # preload the natural_log ACT table set (Square lives in that set too)
    scrln = wpool.tile([1, 1], F32, tag="scrln")
    nc.scalar.activation(
        scrln[:], scrsin[:], AF.Ln, bias=1.0, scale=0.0
    )

    final = wpool.tile([1, 2], F32, tag="final")

    # --- doubling: L1 (site pairs, 4 combos), L2 (quads, 16 combos), bf16
    l1 = [wpool.tile([J, BLOC * 16], F32, tag=f"l1_{g}", name=f"l1_{g}") for g in range(2)]
    l2 = [wpool.tile([J, BLOC * 32], F32, tag=f"l2_{g}", name=f"l2_{g}") for g in range(2)]
    for g in range(2):
        eng = _eng(nc, LENG[g])
        lo = g * 8
        o1all = l1[g][:].rearrange(
            "j (i s t1 t2) -> j i s t1 t2", i=BLOC, s=4, t1=2, t2=2
        )
        for t1 in range(2):
            in1 = (
                cs_v[:, t1, :, lo : lo + 8 : 2]
                .unsqueeze(3)
                .broadcast_to([J, BLOC, 4, 2])
            )
            in2 = cs_v[:, :, :, lo + 1 : lo + 8 : 2].transpose([0, 2, 3, 1])
            o1 = o1all[:, :, :, t1, :]
            eng.tensor_tensor(out=o1, in0=in1, in1=in2, op=ALU.mult)
        l1v = l1[g][:].rearrange("j (i s c) -> j i s c", i=BLOC, s=4, c=4)
        o2all = l2[g][:].rearrange(
            "j (i d q1 q2) -> j i d q1 q2", i=BLOC, d=2, q1=4, q2=4
        )
        for d in range(2):
            in1 = l1v[:, :, 2 * d, :].unsqueeze(3).broadcast_to([J, BLOC, 4, 4])
            in2 = l1v[:, :, 2 * d + 1, :].unsqueeze(2).broadcast_to([J, BLOC, 4, 4])
            o2 = o2all[:, :, d, :, :]
            eng.tensor_tensor(out=o2, in0=in1, in1=in2, op=ALU.mult)

    # =====================================================================
    # regularization path -- Pool + PE, fully overlapped with the heavy math
    fin_r = wpool.tile([J, 34], F32, tag="fin_r")
    nc.gpsimd.tensor_copy(fin_r[:, 0:17], blob[:, 0:17])
    nc.gpsimd.tensor_tensor(
        out=fin_r[:, 17:34], in0=blob[:, 0:17], in1=blob[:, 0:17], op=ALU.mult
    )
    fps = fpool.tile([J, 64], F32, tag="fps")
    fout_r = fps[0:37, 0:34]
    nc.tensor.matmul(fout_r, blob[:, PC_MASK : PC_MASK + 37], fin_r[:])
    ss_part = fps[0:37, 17:34]
    sv = wpool.tile([37, 17], F32, tag="sv")
    nc.vector.tensor_copy(sv[:], fps[0:37, 0:17])
    v1 = wpool.tile([37, 17], F32, tag="v1")
    nc.gpsimd.tensor_tensor(out=v1[:], in0=sv[:], in1=sv[:], op=ALU.mult)
    v2 = wpool.tile([37, 17], F32, tag="v2")
    nc.vector.scalar_tensor_tensor(
        out=v2[:], in0=v1[:],
        scalar=blob[0:37, PC_DVEC : PC_DVEC + 1],
        in1=ss_part, op0=ALU.mult, op1=ALU.subtract,
    )
    v3 = wpool.tile([37, 17], F32, tag="v3")
    nc.gpsimd.tensor_tensor(
        out=v3[:], in0=v2[:],
        in1=blob[0:37, PC_REGW : PC_REGW + 17], op=ALU.mult,
    )
    v4 = wpool.tile([37, 17], F32, tag="v4")
    v5 = wpool.tile([37, 1], F32, tag="v5")
    nc.vector.tensor_scalar(
        out=v4[:], in0=v3[:], scalar1=1.0, scalar2=None,
        op0=ALU.mult, op1=ALU.add, accum_out=v5[:],
    )
    rt = fps[0:1, 40:41]
    nc.tensor.matmul(rt, blob[0:37, PC_MASK : PC_MASK + 1], v5[:])
    nc.vector.tensor_copy(final[0:1, 1:2], rt)
    # =====================================================================

    # --- per-group U build, D matmuls, squares
    fin = wpool.tile([J, BLOC], F32, tag="fin")  # per-b sum-of-squares partials
    amp = wpool.tile([1, BLOC], F32, tag="amp")  # per-b amp = D[0,0]
    assert sum(BGRP) == BLOC
    ts_flat = 0
    i0 = 0
    for c, csz in enumerate(BGRP):
        cw = csz * 256
        uc = [
            upool.tile([J, 1024], MM_DT, tag=f"u{g}", name=f"u_{g}_{c}")
            for g in range(2)
        ]
        for g in range(2):
            eng = _eng(nc, UENG[c][g])
            l2v = l2[g][:].rearrange(
                "j (i d c16) -> j i d c16", i=BLOC, d=2, c16=16
            )
            in1 = (
                l2v[:, i0 : i0 + csz, 0, :]
                .unsqueeze(3)
                .broadcast_to([J, csz, 16, 16])
            )
            in2 = (
                l2v[:, i0 : i0 + csz, 1, :]
                .unsqueeze(2)
                .broadcast_to([J, csz, 16, 16])
            )
            ov = uc[g][:, 0:cw].rearrange(
                "j (i u1 u2) -> j i u1 u2", i=csz, u1=16, u2=16
            )
            eng.tensor_tensor(out=ov, in0=in1, in1=in2, op=ALU.mult)

        # D matmuls for this group into one wide PSUM tile
        pool = pb if c % 2 == 0 else pa
        dt = pool.tile([J, 512 * (4 if pool is pa else 3)], F32, tag="D", name=f"D_{c}")
        for k in range(csz):
            rhs = uc[1][:, k * 256 : (k + 1) * 256]
            for h in range(2):
                lhsT = uc[0][:, k * 256 + h * 128 : k * 256 + (h + 1) * 128]
                nc.tensor.matmul(
                    dt[:, k * 512 + h * 256 : k * 512 + (h + 1) * 256], lhsT, rhs
                )

        # amp[b] = D_b[0,0]: partition 0, col k*512
        nc.vector.tensor_copy(
            amp[0:1, i0 : i0 + csz], dt[0:1, 0 : csz * 512 : 512]
        )

        # squares: first nd b's ACT-direct; rest one wide ACT square + TS per b
        nd = NDIR[c]
        for k in range(nd):
            sl = dt[:, k * 512 : (k + 1) * 512]
            nc.scalar.activation(
                sl, sl, AF.Square, accum_out=fin[:, i0 + k : i0 + k + 1]
            )
        nw = csz - nd
        if nw > 0:
            dsq = qpool.tile([J, 2048], BF16, tag="dsq", name=f"dsq_{c}")
            nc.scalar.activation(
                dsq[:, 0 : nw * 512], dt[:, nd * 512 : csz * 512], AF.Square
            )
            for k in range(nw):
                eng = _eng(nc, TSENG[ts_flat])
                ts_flat += 1
                i = i0 + nd + k
                eng.tensor_scalar(
                    out=dsq[:, k * 512 : (k + 1) * 512],
                    in0=dsq[:, k * 512 : (k + 1) * 512],
                    scalar1=1.0, scalar2=None,
                    op0=ALU.mult, op1=ALU.add,
                    accum_out=fin[:, i : i + 1],
                )
        i0 += csz

    # --- loss tail
    fout = fps[0:1, 48 : 48 + BLOC]
    nc.tensor.matmul(fout, ones_ap, fin[:])
    # tt2 = [amp^2 (16) | norm (16)]; one Ln over 32 lanes
    tt2 = wpool.tile([1, 32], F32, tag="tt2")
    nc.vector.tensor_tensor(
        out=tt2[0:1, 0:BLOC], in0=amp[:], in1=amp[:], op=ALU.mult
    )
    nc.vector.tensor_copy(tt2[0:1, BLOC:32], fout)
    lno = wpool.tile([1, 32], F32, tag="lno")
    nc.scalar.activation(lno[:], tt2[:], AF.Ln)
    # final[0,0] = sum_b (ln norm - ln amp^2) / B
    diff = wpool.tile([1, BLOC], F32, tag="diff")
    nc.vector.tensor_tensor(
        out=diff[:], in0=lno[0:1, BLOC:32], in1=lno[0:1, 0:BLOC], op=ALU.subtract
    )
    scr6 = wpool.tile([1, BLOC], F32, tag="scr6")
    nc.vector.tensor_scalar(
        out=scr6[:], in0=diff[:], scalar1=1.0 / float(B), scalar2=None,
        op0=ALU.mult, op1=ALU.add, accum_out=final[0:1, 0:1],
    )

    nc.sync.dma_start(out_d[:, :], final[:])


def make_in_maps(input_ds, theta, coef):
    input_ds = np.asarray(input_ds, dtype=np.float32)
    theta = np.asarray(theta, dtype=np.float32)
    coef = np.asarray(coef, dtype=np.float32)
    pr = build_blob()
    pr[:, PC_THETA : PC_THETA + L] = theta.transpose(1, 2, 0).reshape(J, L)
    pr[:, PC_COEF] = coef.reshape(J)
    in_maps = []
    for c in range(NCORES):
        blob = pr.copy()
        sl = input_ds[c * BLOC : (c + 1) * BLOC, :].reshape(1, BLOC * L)
        blob[:, PC_IND : PC_IND + BLOC * L] = sl
        in_maps.append({"blob": blob})
    return in_maps


_NC_CACHE = None


def _get_program():
    global _NC_CACHE
    if _NC_CACHE is None:
        _NC_CACHE = build_program()
    return _NC_CACHE


def combine_outputs(results):
    loss = 0.0
    for c in range(NCORES):
        loss += float(results[c]["out"][0, 0])
    loss += float(results[0]["out"][0, 1])
    return np.float32(loss)


def kernel(input_ds, theta, coef):
    from concourse.bass_utils import run_bass_kernel_spmd

    nc = _get_program()
    in_maps = make_in_maps(input_ds, theta, coef)
    res = run_bass_kernel_spmd(nc, in_maps, core_ids=list(range(NCORES)))
    return combine_outputs(res.results)


# revision 10
# speedup vs baseline: 1.0234x; 1.0234x over previous
"""Trainium2 Bass kernel for the MANTIS quantum-circuit-loss nn.Module.

Shapes (hardcoded): B=128, L=16, M=32, P=4.  8 NeuronCores, batch-sharded
(16 batch elements per core).

Math
----
Let j = (m, p) flattened (M*P = 128 == partition count) and
    A[b, l, j] = theta[l, j] + scal[p(j)] * input_ds[b, l]
    CA = cos(A), SA = sin(A)                       (ACT Sin + pi/2 bias)

prob term:      amp[b]  = sum_j coef_j prod_l CA[b,l,j]
normalization:  norm[b] = sum_{j,k} coef_j coef_k prod_l cos(A[b,l,j]-A[b,l,k])

norm[b] is the squared norm of a sum of 128 product states in the 2^16-dim
site space.  Split the 16 sites into two groups of 8; per group build the
256 branch-product vectors U_g[j, T] by log-doubling (bf16 elementwise
multiplies, split across DVE and Pool).  With coef folded into U0 (site 0):
    D_b[T1, T2] = sum_j (c U0)[j, T1] U1[j, T2]    (PE matmul, K=128, bf16)
    norm[b] = ||D_b||^2      amp[b] = D_b[0, 0]    (read from PSUM directly)
    loss_b  = (ln(norm) - ln(amp^2)) / B           (EPS term negligible)

Squares: batch elements are grouped [2,4,3,4,3] into wide PSUM tiles; ACT
does one Square pass per group (PSUM f32 -> SBUF bf16), then one DVE
tensor_scalar per element accumulates into fin (bf16 4x mode).  The final
partition reduction is one ones-matmul.  Regularization variances run on
Pool + PE, fully overlapped.  Each core returns [1,2]:
    out[0,0] = (1/128) * sum_{local b} (ln norm_b - ln amp_b^2)
    out[0,1] = REG_C*var(coef) + REG_THETA_M*... + REG_THETA_P*...
Host combine: loss = sum_c out_c[0,0] + out_0[0,1].
"""

import os

import numpy as np

import concourse.bacc as bacc
import concourse.bass as bass
import concourse.mybir as mybir
import concourse.tile as tile

B, L, M, P = 128, 16, 32, 4
NCORES = 8
BLOC = B // NCORES  # 16 batch elements per core
J = M * P  # 128
REG_C = 0.01
REG_THETA_M = 0.01
REG_THETA_P = 0.01

F32 = mybir.dt.float32
BF16 = mybir.dt.bfloat16
AF = mybir.ActivationFunctionType
ALU = mybir.AluOpType

MM_DT = BF16 if os.environ.get("MANTIS_MM_DT", "f32r") == "bf16" else mybir.dt.float32r

# batch grouping into PSUM tiles: [2,4,3,4,3] alternating pools B,A,B,A,B
BGRP = [int(x) for x in os.environ.get("MANTIS_BGRP", "2,4,3,4,3").split(",")]
# per group: how many leading b's use ACT-direct square+accum (rest: wide+TS)
NDIR = [int(x) for x in os.environ.get("MANTIS_NDIR", "1,2,2,2,2").split(",")]
# per group: engine for the group's U-build, one char per site-group:
#   'd' = DVE, 'p' = Pool
UENG = os.environ.get("MANTIS_UENG", "dp,dp,dp,dp,dp").split(",")
# engine for the TS square-reduce per batch element (flat list over wide b's)
TSENG = os.environ.get("MANTIS_TSENG", "d" * 16)
# engine for L1/L2 doubling per site-group
LENG = os.environ.get("MANTIS_LENG", "dp")

# input blob column layout
PC_THETA = 0  # 16 cols: theta_t[j, l]
PC_COEF = 16  # 1 col
PC_SCAL = 17  # 1 col: pi / 2^(p(j)+1)
PC_DVEC = 18  # 1 col: 1/n for the var terms (rows 0:37)
PC_HALFPI = 19  # 1 col: pi/2 (ACT bias for cos-via-sin)
PC_IND = 20  # 256 cols: input_ds slice, [i, l]
PC_MASK = 276  # 37 cols: [ones | mask_p(4) | mask_m(32)]
PC_REGW = 313  # 17 cols: reg weights (rows 0:37)
IN_COLS = 330


def build_blob() -> np.ndarray:
    pr = np.zeros((J, IN_COLS), dtype=np.float32)
    sf = (np.pi / 2.0 ** (np.arange(P) + 1.0)).astype(np.float32)
    pr[:, PC_SCAL] = np.tile(sf, M)
    pr[0, PC_DVEC] = 1.0 / 128.0
    pr[1:5, PC_DVEC] = 1.0 / 32.0
    pr[5:37, PC_DVEC] = 1.0 / 4.0
    pr[:, PC_HALFPI] = np.pi / 2.0
    pr[:, PC_MASK] = 1.0  # ones
    jj = np.arange(J)
    pr[jj, PC_MASK + 1 + (jj % 4)] = 1.0  # mask_p
    pr[jj, PC_MASK + 5 + (jj // 4)] = 1.0  # mask_m
    # REGW rows 0:37: weight for each cell of (S^2/n - SS) so that
    # sum(REGW * (S^2/n - SS)) == reg_total; var = (SS - S^2/n)/(n-1).
    pr[0, PC_REGW + 16] = -REG_C / 127.0
    pr[1:5, PC_REGW : PC_REGW + 16] = -REG_THETA_M / 64.0 / 31.0
    pr[5:37, PC_REGW : PC_REGW + 16] = -REG_THETA_P / 512.0 / 3.0
    return pr


def build_program():
    nc = bacc.Bacc(
        "TRN2",
        target_bir_lowering=False,
        debug=False,
        num_devices=NCORES,
    )
    blob_d = nc.dram_tensor("blob", [J, IN_COLS], F32, kind="ExternalInput")
    out_d = nc.dram_tensor("out", [1, 34], F32, kind="ExternalOutput")

    with tile.TileContext(nc) as tc:
        with (
            tc.tile_pool(name="const", bufs=1) as cpool,
            tc.tile_pool(name="work", bufs=1) as wpool,
            tc.tile_pool(name="upl", bufs=10) as upool,
            tc.tile_pool(name="sq", bufs=2) as qpool,
            tc.tile_pool(name="pa", bufs=1, space=bass.MemorySpace.PSUM) as pa,
            tc.tile_pool(name="pb", bufs=1, space=bass.MemorySpace.PSUM) as pb,
            tc.tile_pool(name="fps", bufs=1, space=bass.MemorySpace.PSUM) as fpool,
        ):
            _emit(nc, tc, cpool, wpool, upool, qpool, pa, pb, fpool, blob_d, out_d)
    nc.compile()
    return nc


def _eng(nc, c):
    return nc.vector if c == "d" else nc.gpsimd


def _emit(nc, tc, cpool, wpool, upool, qpool, pa, pb, fpool, blob_d, out_d):
    blob = cpool.tile([J, IN_COLS], F32, tag="blob")
    # critical head (params + inds) first; masks/regw second
    nc.sync.dma_start(blob[:, 0:PC_MASK], blob_d[:, 0:PC_MASK])
    nc.sync.dma_start(blob[:, PC_MASK:], blob_d[:, PC_MASK:])

    # warm the Sin ACT table while the input DMA is in flight
    scrsin = wpool.tile([1, 1], F32, tag="scrsin")
    nc.gpsimd.memset(scrsin[:], 0.0)
    scrsin2 = wpool.tile([1, 1], F32, tag="scrsin2")
    nc.scalar.activation(scrsin2[:], scrsin[:], AF.Sin)

    theta_ap = blob[:, PC_THETA : PC_THETA + L]
    coef_ap = blob[:, PC_COEF : PC_COEF + 1]
    scal_ap = blob[:, PC_SCAL : PC_SCAL + 1]
    ones_ap = blob[:, PC_MASK : PC_MASK + 1]

    # --- stage A: ARG[j, (i,l)] = theta[j,l] + scal[j]*inds[i,l]
    arg = wpool.tile([J, BLOC * L], F32, tag="arg")
    in_bc = blob[:, PC_IND : PC_IND + BLOC * L].rearrange(
        "j (i l) -> j i l", i=BLOC, l=L
    )
    th_bc = theta_ap.unsqueeze(1).broadcast_to([J, BLOC, L])
    arg_v = arg[:].rearrange("j (i l) -> j i l", i=BLOC, l=L)
    nc.vector.scalar_tensor_tensor(
        out=arg_v, in0=in_bc, scalar=scal_ap, in1=th_bc,
        op0=ALU.mult, op1=ALU.add,
    )

    # --- CS[j, (t,i,l)] in bf16: t=0 -> cos(A), t=1 -> sin(A)
    cs = wpool.tile([J, 2 * BLOC * L], F32, tag="cs")
    nc.scalar.activation(
        cs[:, 0 : BLOC * L], arg[:], AF.Sin,
        bias=blob[:, PC_HALFPI : PC_HALFPI + 1], scale=-1.0,
    )
    nc.scalar.activation(cs[:, BLOC * L : 2 * BLOC * L], arg[:], AF.Sin)

    cs_v = cs[:].rearrange("j (t i l) -> j t i l", t=2, i=BLOC, l=L)
    # fold coef into site l=0 (both branches)
    nc.vector.tensor_scalar_mul(cs_v[:, :, :, 0:1], cs_v[:, :, :, 0:1], coef_ap)

    final = wpool.tile([1, 34], F32, tag="final")

    # --- doubling: L1 (site pairs, 4 combos), L2 (quads, 16 combos), bf16
    l1 = [wpool.tile([J, BLOC * 16], F32, tag=f"l1_{g}", name=f"l1_{g}") for g in range(2)]
    l2 = [wpool.tile([J, BLOC * 32], F32, tag=f"l2_{g}", name=f"l2_{g}") for g in range(2)]
    for g in range(2):
        eng = _eng(nc, LENG[g])
        lo = g * 8
        o1all = l1[g][:].rearrange(
            "j (i s t1 t2) -> j i s t1 t2", i=BLOC, s=4, t1=2, t2=2
        )
        for t1 in range(2):
            in1 = (
                cs_v[:, t1, :, lo : lo + 8 : 2]
                .unsqueeze(3)
                .broadcast_to([J, BLOC, 4, 2])
            )
            in2 = cs_v[:, :, :, lo + 1 : lo + 8 : 2].transpose([0, 2, 3, 1])
            o1 = o1all[:, :, :, t1, :]
            eng.tensor_tensor(out=o1, in0=in1, in1=in2, op=ALU.mult)
        l1v = l1[g][:].rearrange("j (i s c) -> j i s c", i=BLOC, s=4, c=4)
        o2all = l2[g][:].rearrange(
            "j (i d q1 q2) -> j i d q1 q2", i=BLOC, d=2, q1=4, q2=4
        )
        for d in range(2):
            in1 = l1v[:, :, 2 * d, :].unsqueeze(3).broadcast_to([J, BLOC, 4, 4])
            in2 = l1v[:, :, 2 * d + 1, :].unsqueeze(2).broadcast_to([J, BLOC, 4, 4])
            o2 = o2all[:, :, d, :, :]
            eng.tensor_tensor(out=o2, in0=in1, in1=in2, op=ALU.mult)

    # =====================================================================
    # regularization path -- Pool + PE, fully overlapped with the heavy math
    fin_r = wpool.tile([J, 34], F32, tag="fin_r")
    nc.gpsimd.tensor_copy(fin_r[:, 0:17], blob[:, 0:17])
    nc.gpsimd.tensor_tensor(
        out=fin_r[:, 17:34], in0=blob[:, 0:17], in1=blob[:, 0:17], op=ALU.mult
    )
    fps = fpool.tile([J, 64], F32, tag="fps")
    fout_r = fps[0:37, 0:34]
    nc.tensor.matmul(fout_r, blob[:, PC_MASK : PC_MASK + 37], fin_r[:])
    ss_part = fps[0:37, 17:34]
    sv = wpool.tile([37, 17], F32, tag="sv")
    nc.vector.tensor_copy(sv[:], fps[0:37, 0:17])
    v1 = wpool.tile([37, 17], F32, tag="v1")
    nc.gpsimd.tensor_tensor(out=v1[:], in0=sv[:], in1=sv[:], op=ALU.mult)
    v2 = wpool.tile([37, 17], F32, tag="v2")
    nc.vector.scalar_tensor_tensor(
        out=v2[:], in0=v1[:],
        scalar=blob[0:37, PC_DVEC : PC_DVEC + 1],
        in1=ss_part, op0=ALU.mult, op1=ALU.subtract,
    )
    v3 = wpool.tile([37, 17], F32, tag="v3")
    nc.gpsimd.tensor_tensor(
        out=v3[:], in0=v2[:],
        in1=blob[0:37, PC_REGW : PC_REGW + 17], op=ALU.mult,
    )
    v4 = wpool.tile([37, 17], F32, tag="v4")
    v5 = wpool.tile([37, 1], F32, tag="v5")
    nc.vector.tensor_scalar(
        out=v4[:], in0=v3[:], scalar1=1.0, scalar2=None,
        op0=ALU.mult, op1=ALU.add, accum_out=v5[:],
    )
    rt = fps[0:1, 40:41]
    nc.tensor.matmul(rt, blob[0:37, PC_MASK : PC_MASK + 1], v5[:])
    nc.vector.tensor_copy(final[0:1, 32:33], rt)
    # =====================================================================

    # --- per-group U build, D matmuls, squares
    fin = wpool.tile([J, BLOC], F32, tag="fin")  # per-b sum-of-squares partials
    assert sum(BGRP) == BLOC
    ts_flat = 0
    i0 = 0
    for c, csz in enumerate(BGRP):
        cw = csz * 256
        uc = [
            upool.tile([J, 1024], MM_DT, tag=f"u{g}", name=f"u_{g}_{c}")
            for g in range(2)
        ]
        for g in range(2):
            eng = _eng(nc, UENG[c][g])
            l2v = l2[g][:].rearrange(
                "j (i d c16) -> j i d c16", i=BLOC, d=2, c16=16
            )
            in1 = (
                l2v[:, i0 : i0 + csz, 0, :]
                .unsqueeze(3)
                .broadcast_to([J, csz, 16, 16])
            )
            in2 = (
                l2v[:, i0 : i0 + csz, 1, :]
                .unsqueeze(2)
                .broadcast_to([J, csz, 16, 16])
            )
            ov = uc[g][:, 0:cw].rearrange(
                "j (i u1 u2) -> j i u1 u2", i=csz, u1=16, u2=16
            )
            eng.tensor_tensor(out=ov, in0=in1, in1=in2, op=ALU.mult)

        # D matmuls for this group into one wide PSUM tile
        pool = pb if c % 2 == 0 else pa
        dt = pool.tile([J, 512 * (4 if pool is pa else 3)], F32, tag="D", name=f"D_{c}")
        for k in range(csz):
            rhs = uc[1][:, k * 256 : (k + 1) * 256]
            for h in range(2):
                lhsT = uc[0][:, k * 256 + h * 128 : k * 256 + (h + 1) * 128]
                nc.tensor.matmul(
                    dt[:, k * 512 + h * 256 : k * 512 + (h + 1) * 256], lhsT, rhs
                )

        # amp[b] = D_b[0,0]: partition 0, col k*512
        nc.vector.tensor_copy(
            final[0:1, BLOC + i0 : BLOC + i0 + csz], dt[0:1, 0 : csz * 512 : 512]
        )

        # squares: first nd b's ACT-direct; rest one wide ACT square + TS per b
        nd = NDIR[c]
        for k in range(nd):
            sl = dt[:, k * 512 : (k + 1) * 512]
            nc.scalar.activation(
                sl, sl, AF.Square, accum_out=fin[:, i0 + k : i0 + k + 1]
            )
        nw = csz - nd
        if nw > 0:
            dsq = qpool.tile([J, 2048], F32, tag="dsq", name=f"dsq_{c}")
            nc.scalar.activation(
                dsq[:, 0 : nw * 512], dt[:, nd * 512 : csz * 512], AF.Square
            )
            for k in range(nw):
                eng = _eng(nc, TSENG[ts_flat])
                ts_flat += 1
                i = i0 + nd + k
                eng.tensor_scalar(
                    out=dsq[:, k * 512 : (k + 1) * 512],
                    in0=dsq[:, k * 512 : (k + 1) * 512],
                    scalar1=1.0, scalar2=None,
                    op0=ALU.mult, op1=ALU.add,
                    accum_out=fin[:, i : i + 1],
                )
        i0 += csz

    # --- tail: norms to final[0:16]; amps already collected in final[16:32]
    fout = fps[0:1, 48 : 48 + BLOC]
    nc.tensor.matmul(fout, ones_ap, fin[:])
    nc.vector.tensor_copy(final[0:1, 0:BLOC], fout)

    nc.sync.dma_start(out_d[:, :], final[:])


def make_in_maps(input_ds, theta, coef):
    input_ds = np.asarray(input_ds, dtype=np.float32)
    theta = np.asarray(theta, dtype=np.float32)
    coef = np.asarray(coef, dtype=np.float32)
    pr = build_blob()
    pr[:, PC_THETA : PC_THETA + L] = theta.transpose(1, 2, 0).reshape(J, L)
    pr[:, PC_COEF] = coef.reshape(J)
    in_maps = []
    for c in range(NCORES):
        blob = pr.copy()
        sl = input_ds[c * BLOC : (c + 1) * BLOC, :].reshape(1, BLOC * L)
        blob[:, PC_IND : PC_IND + BLOC * L] = sl
        in_maps.append({"blob": blob})
    return in_maps


_NC_CACHE = None


def _get_program():
    global _NC_CACHE
    if _NC_CACHE is None:
        _NC_CACHE = build_program()
    return _NC_CACHE


def combine_outputs(results):
    loss = 0.0
    for c in range(NCORES):
        o = results[c]["out"][0]
        norms = o[0:BLOC].astype(np.float64)
        amps = o[BLOC : 2 * BLOC].astype(np.float64)
        loss += float(np.sum(np.log(norms) - np.log(amps * amps)) / B)
    loss += float(results[0]["out"][0, 32])
    return np.float32(loss)


def kernel(input_ds, theta, coef):
    from concourse.bass_utils import run_bass_kernel_spmd

    nc = _get_program()
    in_maps = make_in_maps(input_ds, theta, coef)
    res = run_bass_kernel_spmd(nc, in_maps, core_ids=list(range(NCORES)))
    return combine_outputs(res.results)


# revision 11
# speedup vs baseline: 1.1463x; 1.1201x over previous
"""Trainium2 Bass kernel for the MANTIS quantum-circuit-loss nn.Module.

Shapes (hardcoded): B=128, L=16, M=32, P=4.  8 NeuronCores, batch-sharded
(16 batch elements per core).

Math
----
Let j = (m, p) flattened (M*P = 128 == partition count) and
    A[b, l, j] = theta[l, j] + scal[p(j)] * input_ds[b, l]
    CA = cos(A), SA = sin(A)                       (ACT Sin + pi/2 bias)

prob term:      amp[b]  = sum_j coef_j prod_l CA[b,l,j]
normalization:  norm[b] = sum_{j,k} coef_j coef_k prod_l cos(A[b,l,j]-A[b,l,k])

norm[b] is the squared norm of a sum of 128 product states in the 2^16-dim
site space.  Split the 16 sites into two groups of 8; per group build the
256 branch-product vectors U_g[j, T] by log-doubling (bf16 elementwise
multiplies, split across DVE and Pool).  With coef folded into U0 (site 0):
    D_b[T1, T2] = sum_j (c U0)[j, T1] U1[j, T2]    (PE matmul, K=128, bf16)
    norm[b] = ||D_b||^2      amp[b] = D_b[0, 0]    (read from PSUM directly)
    loss_b  = (ln(norm) - ln(amp^2)) / B           (EPS term negligible)

Squares: batch elements are grouped [2,4,3,4,3] into wide PSUM tiles; ACT
does one Square pass per group (PSUM f32 -> SBUF bf16), then one DVE
tensor_scalar per element accumulates into fin (bf16 4x mode).  The final
partition reduction is one ones-matmul.  Regularization variances run on
Pool + PE, fully overlapped.  Each core returns [1,2]:
    out[0,0] = (1/128) * sum_{local b} (ln norm_b - ln amp_b^2)
    out[0,1] = REG_C*var(coef) + REG_THETA_M*... + REG_THETA_P*...
Host combine: loss = sum_c out_c[0,0] + out_0[0,1].
"""

import os

import numpy as np

import concourse.bacc as bacc
import concourse.bass as bass
import concourse.mybir as mybir
import concourse.tile as tile

B, L, M, P = 128, 16, 32, 4
NCORES = 8
BLOC = B // NCORES  # 16 batch elements per core
J = M * P  # 128
REG_C = 0.01
REG_THETA_M = 0.01
REG_THETA_P = 0.01

F32 = mybir.dt.float32
BF16 = mybir.dt.bfloat16
AF = mybir.ActivationFunctionType
ALU = mybir.AluOpType

MM_DT = BF16 if os.environ.get("MANTIS_MM_DT", "f32r") == "bf16" else mybir.dt.float32r

# batch grouping into PSUM tiles: [2,4,3,4,3] alternating pools B,A,B,A,B
BGRP = [int(x) for x in os.environ.get("MANTIS_BGRP", "2,4,3,4,3").split(",")]
# per group: how many leading b's use ACT-direct square+accum (rest: wide+TS)
NDIR = [int(x) for x in os.environ.get("MANTIS_NDIR", "1,3,2,3,3").split(",")]
# per group: engine for the group's U-build, one char per site-group:
#   'd' = DVE, 'p' = Pool  (Pool measured net-negative: SBUF contention)
UENG = os.environ.get("MANTIS_UENG", "dd,dd,dd,dd,dd").split(",")
# engine for the TS square-reduce per batch element (flat list over wide b's)
TSENG = os.environ.get("MANTIS_TSENG", "d" * 16)
# engine for L1/L2 doubling per site-group
LENG = os.environ.get("MANTIS_LENG", "dd")

# input blob column layout
PC_THETA = 0  # 16 cols: theta_t[j, l]
PC_COEF = 16  # 1 col
PC_SCAL = 17  # 1 col: pi / 2^(p(j)+1)
PC_DVEC = 18  # 1 col: 1/n for the var terms (rows 0:37)
PC_HALFPI = 19  # 1 col: pi/2 (ACT bias for cos-via-sin)
PC_IND = 20  # 256 cols: input_ds slice, [i, l]
PC_MASK = 276  # 37 cols: [ones | mask_p(4) | mask_m(32)]
PC_REGW = 313  # 17 cols: reg weights (rows 0:37)
IN_COLS = 330


def build_blob() -> np.ndarray:
    pr = np.zeros((J, IN_COLS), dtype=np.float32)
    sf = (np.pi / 2.0 ** (np.arange(P) + 1.0)).astype(np.float32)
    pr[:, PC_SCAL] = np.tile(sf, M)
    pr[0, PC_DVEC] = 1.0 / 128.0
    pr[1:5, PC_DVEC] = 1.0 / 32.0
    pr[5:37, PC_DVEC] = 1.0 / 4.0
    pr[:, PC_HALFPI] = np.pi / 2.0
    pr[:, PC_MASK] = 1.0  # ones
    jj = np.arange(J)
    pr[jj, PC_MASK + 1 + (jj % 4)] = 1.0  # mask_p
    pr[jj, PC_MASK + 5 + (jj // 4)] = 1.0  # mask_m
    # REGW rows 0:37: weight for each cell of (S^2/n - SS) so that
    # sum(REGW * (S^2/n - SS)) == reg_total; var = (SS - S^2/n)/(n-1).
    pr[0, PC_REGW + 16] = -REG_C / 127.0
    pr[1:5, PC_REGW : PC_REGW + 16] = -REG_THETA_M / 64.0 / 31.0
    pr[5:37, PC_REGW : PC_REGW + 16] = -REG_THETA_P / 512.0 / 3.0
    return pr


def build_program():
    nc = bacc.Bacc(
        "TRN2",
        target_bir_lowering=False,
        debug=False,
        num_devices=NCORES,
    )
    blob_d = nc.dram_tensor("blob", [J, IN_COLS], F32, kind="ExternalInput")
    out_d = nc.dram_tensor("out", [1, 34], F32, kind="ExternalOutput")

    with tile.TileContext(nc) as tc:
        with (
            tc.tile_pool(name="const", bufs=1) as cpool,
            tc.tile_pool(name="work", bufs=1) as wpool,
            tc.tile_pool(name="upl", bufs=10) as upool,
            tc.tile_pool(name="sq", bufs=2) as qpool,
            tc.tile_pool(name="pa", bufs=1, space=bass.MemorySpace.PSUM) as pa,
            tc.tile_pool(name="pb", bufs=1, space=bass.MemorySpace.PSUM) as pb,
            tc.tile_pool(name="fps", bufs=1, space=bass.MemorySpace.PSUM) as fpool,
        ):
            _emit(nc, tc, cpool, wpool, upool, qpool, pa, pb, fpool, blob_d, out_d)
    nc.compile()
    return nc


def _eng(nc, c):
    return nc.vector if c == "d" else nc.gpsimd


def _emit(nc, tc, cpool, wpool, upool, qpool, pa, pb, fpool, blob_d, out_d):
    blob = cpool.tile([J, IN_COLS], F32, tag="blob")
    # critical head (params + inds) first; masks/regw second
    nc.sync.dma_start(blob[:, 0:PC_MASK], blob_d[:, 0:PC_MASK])
    nc.sync.dma_start(blob[:, PC_MASK:], blob_d[:, PC_MASK:])

    # warm the Sin ACT table while the input DMA is in flight
    scrsin = wpool.tile([1, 1], F32, tag="scrsin")
    nc.gpsimd.memset(scrsin[:], 0.0)
    scrsin2 = wpool.tile([1, 1], F32, tag="scrsin2")
    nc.scalar.activation(scrsin2[:], scrsin[:], AF.Sin)

    theta_ap = blob[:, PC_THETA : PC_THETA + L]
    coef_ap = blob[:, PC_COEF : PC_COEF + 1]
    scal_ap = blob[:, PC_SCAL : PC_SCAL + 1]
    ones_ap = blob[:, PC_MASK : PC_MASK + 1]

    # --- stage A: ARG[j, (i,l)] = theta[j,l] + scal[j]*inds[i,l]
    arg = wpool.tile([J, BLOC * L], F32, tag="arg")
    in_bc = blob[:, PC_IND : PC_IND + BLOC * L].rearrange(
        "j (i l) -> j i l", i=BLOC, l=L
    )
    th_bc = theta_ap.unsqueeze(1).broadcast_to([J, BLOC, L])
    arg_v = arg[:].rearrange("j (i l) -> j i l", i=BLOC, l=L)
    nc.vector.scalar_tensor_tensor(
        out=arg_v, in0=in_bc, scalar=scal_ap, in1=th_bc,
        op0=ALU.mult, op1=ALU.add,
    )

    # --- CS[j, (t,i,l)] in bf16: t=0 -> cos(A), t=1 -> sin(A)
    cs = wpool.tile([J, 2 * BLOC * L], F32, tag="cs")
    nc.scalar.activation(
        cs[:, 0 : BLOC * L], arg[:], AF.Sin,
        bias=blob[:, PC_HALFPI : PC_HALFPI + 1], scale=-1.0,
    )
    nc.scalar.activation(cs[:, BLOC * L : 2 * BLOC * L], arg[:], AF.Sin)

    cs_v = cs[:].rearrange("j (t i l) -> j t i l", t=2, i=BLOC, l=L)
    # fold coef into site l=0 (both branches)
    nc.vector.tensor_scalar_mul(cs_v[:, :, :, 0:1], cs_v[:, :, :, 0:1], coef_ap)

    final = wpool.tile([1, 34], F32, tag="final")

    # --- doubling: L1 (site pairs, 4 combos), L2 (quads, 16 combos), bf16
    l1 = [wpool.tile([J, BLOC * 16], F32, tag=f"l1_{g}", name=f"l1_{g}") for g in range(2)]
    l2 = [wpool.tile([J, BLOC * 32], F32, tag=f"l2_{g}", name=f"l2_{g}") for g in range(2)]
    for g in range(2):
        eng = _eng(nc, LENG[g])
        lo = g * 8
        o1all = l1[g][:].rearrange(
            "j (i s t1 t2) -> j i s t1 t2", i=BLOC, s=4, t1=2, t2=2
        )
        for t1 in range(2):
            in1 = (
                cs_v[:, t1, :, lo : lo + 8 : 2]
                .unsqueeze(3)
                .broadcast_to([J, BLOC, 4, 2])
            )
            in2 = cs_v[:, :, :, lo + 1 : lo + 8 : 2].transpose([0, 2, 3, 1])
            o1 = o1all[:, :, :, t1, :]
            eng.tensor_tensor(out=o1, in0=in1, in1=in2, op=ALU.mult)
        l1v = l1[g][:].rearrange("j (i s c) -> j i s c", i=BLOC, s=4, c=4)
        o2all = l2[g][:].rearrange(
            "j (i d q1 q2) -> j i d q1 q2", i=BLOC, d=2, q1=4, q2=4
        )
        for d in range(2):
            in1 = l1v[:, :, 2 * d, :].unsqueeze(3).broadcast_to([J, BLOC, 4, 4])
            in2 = l1v[:, :, 2 * d + 1, :].unsqueeze(2).broadcast_to([J, BLOC, 4, 4])
            o2 = o2all[:, :, d, :, :]
            eng.tensor_tensor(out=o2, in0=in1, in1=in2, op=ALU.mult)

    # =====================================================================
    # regularization path -- Pool + PE, fully overlapped with the heavy math
    fin_r = wpool.tile([J, 34], F32, tag="fin_r")
    nc.gpsimd.tensor_copy(fin_r[:, 0:17], blob[:, 0:17])
    nc.gpsimd.tensor_tensor(
        out=fin_r[:, 17:34], in0=blob[:, 0:17], in1=blob[:, 0:17], op=ALU.mult
    )
    fps = fpool.tile([J, 64], F32, tag="fps")
    fout_r = fps[0:37, 0:34]
    nc.tensor.matmul(fout_r, blob[:, PC_MASK : PC_MASK + 37], fin_r[:])
    ss_part = fps[0:37, 17:34]
    sv = wpool.tile([37, 17], F32, tag="sv")
    nc.vector.tensor_copy(sv[:], fps[0:37, 0:17])
    v1 = wpool.tile([37, 17], F32, tag="v1")
    nc.gpsimd.tensor_tensor(out=v1[:], in0=sv[:], in1=sv[:], op=ALU.mult)
    v2 = wpool.tile([37, 17], F32, tag="v2")
    nc.vector.scalar_tensor_tensor(
        out=v2[:], in0=v1[:],
        scalar=blob[0:37, PC_DVEC : PC_DVEC + 1],
        in1=ss_part, op0=ALU.mult, op1=ALU.subtract,
    )
    v3 = wpool.tile([37, 17], F32, tag="v3")
    nc.gpsimd.tensor_tensor(
        out=v3[:], in0=v2[:],
        in1=blob[0:37, PC_REGW : PC_REGW + 17], op=ALU.mult,
    )
    v4 = wpool.tile([37, 17], F32, tag="v4")
    v5 = wpool.tile([37, 1], F32, tag="v5")
    nc.vector.tensor_scalar(
        out=v4[:], in0=v3[:], scalar1=1.0, scalar2=None,
        op0=ALU.mult, op1=ALU.add, accum_out=v5[:],
    )
    rt = fps[0:1, 40:41]
    nc.tensor.matmul(rt, blob[0:37, PC_MASK : PC_MASK + 1], v5[:])
    nc.vector.tensor_copy(final[0:1, 32:33], rt)
    # =====================================================================

    # --- per-group U build, D matmuls, squares
    fin = wpool.tile([J, BLOC], F32, tag="fin")  # per-b sum-of-squares partials
    assert sum(BGRP) == BLOC
    ts_flat = 0
    i0 = 0
    for c, csz in enumerate(BGRP):
        cw = csz * 256
        uc = [
            upool.tile([J, 1024], MM_DT, tag=f"u{g}", name=f"u_{g}_{c}")
            for g in range(2)
        ]
        for g in range(2):
            eng = _eng(nc, UENG[c][g])
            l2v = l2[g][:].rearrange(
                "j (i d c16) -> j i d c16", i=BLOC, d=2, c16=16
            )
            in1 = (
                l2v[:, i0 : i0 + csz, 0, :]
                .unsqueeze(3)
                .broadcast_to([J, csz, 16, 16])
            )
            in2 = (
                l2v[:, i0 : i0 + csz, 1, :]
                .unsqueeze(2)
                .broadcast_to([J, csz, 16, 16])
            )
            ov = uc[g][:, 0:cw].rearrange(
                "j (i u1 u2) -> j i u1 u2", i=csz, u1=16, u2=16
            )
            eng.tensor_tensor(out=ov, in0=in1, in1=in2, op=ALU.mult)

        # D matmuls for this group into one wide PSUM tile
        pool = pb if c % 2 == 0 else pa
        dt = pool.tile([J, 512 * (4 if pool is pa else 3)], F32, tag="D", name=f"D_{c}")
        for k in range(csz):
            rhs = uc[1][:, k * 256 : (k + 1) * 256]
            for h in range(2):
                lhsT = uc[0][:, k * 256 + h * 128 : k * 256 + (h + 1) * 128]
                nc.tensor.matmul(
                    dt[:, k * 512 + h * 256 : k * 512 + (h + 1) * 256], lhsT, rhs
                )

        # amp[b] = D_b[0,0]: partition 0, col k*512
        nc.vector.tensor_copy(
            final[0:1, BLOC + i0 : BLOC + i0 + csz], dt[0:1, 0 : csz * 512 : 512]
        )

        # squares: first nd b's ACT-direct; rest one wide ACT square + TS per b
        nd = NDIR[c]
        for k in range(nd):
            sl = dt[:, k * 512 : (k + 1) * 512]
            nc.scalar.activation(
                sl, sl, AF.Square, accum_out=fin[:, i0 + k : i0 + k + 1]
            )
        nw = csz - nd
        if nw > 0:
            dsq = qpool.tile([J, 2048], F32, tag="dsq", name=f"dsq_{c}")
            nc.scalar.activation(
                dsq[:, 0 : nw * 512], dt[:, nd * 512 : csz * 512], AF.Square
            )
            for k in range(nw):
                eng = _eng(nc, TSENG[ts_flat])
                ts_flat += 1
                i = i0 + nd + k
                eng.tensor_scalar(
                    out=dsq[:, k * 512 : (k + 1) * 512],
                    in0=dsq[:, k * 512 : (k + 1) * 512],
                    scalar1=1.0, scalar2=None,
                    op0=ALU.mult, op1=ALU.add,
                    accum_out=fin[:, i : i + 1],
                )
        i0 += csz

    # --- tail: norms to final[0:16]; amps already collected in final[16:32]
    fout = fps[0:1, 48 : 48 + BLOC]
    nc.tensor.matmul(fout, ones_ap, fin[:])
    nc.vector.tensor_copy(final[0:1, 0:BLOC], fout)

    nc.sync.dma_start(out_d[:, :], final[:])


def make_in_maps(input_ds, theta, coef):
    input_ds = np.asarray(input_ds, dtype=np.float32)
    theta = np.asarray(theta, dtype=np.float32)
    coef = np.asarray(coef, dtype=np.float32)
    pr = build_blob()
    pr[:, PC_THETA : PC_THETA + L] = theta.transpose(1, 2, 0).reshape(J, L)
    pr[:, PC_COEF] = coef.reshape(J)
    in_maps = []
    for c in range(NCORES):
        blob = pr.copy()
        sl = input_ds[c * BLOC : (c + 1) * BLOC, :].reshape(1, BLOC * L)
        blob[:, PC_IND : PC_IND + BLOC * L] = sl
        in_maps.append({"blob": blob})
    return in_maps


_NC_CACHE = None


def _get_program():
    global _NC_CACHE
    if _NC_CACHE is None:
        _NC_CACHE = build_program()
    return _NC_CACHE


def combine_outputs(results):
    loss = 0.0
    for c in range(NCORES):
        o = results[c]["out"][0]
        norms = o[0:BLOC].astype(np.float64)
        amps = o[BLOC : 2 * BLOC].astype(np.float64)
        loss += float(np.sum(np.log(norms) - np.log(amps * amps)) / B)
    loss += float(results[0]["out"][0, 32])
    return np.float32(loss)


def kernel(input_ds, theta, coef):
    from concourse.bass_utils import run_bass_kernel_spmd

    nc = _get_program()
    in_maps = make_in_maps(input_ds, theta, coef)
    res = run_bass_kernel_spmd(nc, in_maps, core_ids=list(range(NCORES)))
    return combine_outputs(res.results)
